# revision 5
# baseline (speedup 1.0000x reference)
"""Causal single-head attention (B=256, T=256, C=384, H=64) on 8 trn2 cores.

Data-parallel over batch: each core computes bpc=32 batches independently.

v8 = v2's memory discipline (separate q/k projections so q,k land on PSUM
partitions 0:64 with NO cross-partition copies; dedicated PSUM banks per
tensor; ones-column rowsums in the AV matmul) + pair fusion (2 batches per
emission step amortize per-instruction engine setup costs) + explicit
software pipelining of the emission order. Engines execute their streams in
order, so per-engine order determines cross-pair overlap. Per step i:

  PE : proj(i) 18mm | AV(i-1) 6mm (covers copy latency) | wei(i) 4mm
  ACT: vcopy(i) | exp(i, b0) | exp(i, b1)
  DVE: qkcopy(i) | recip(i-1), scale(i-1)
  Pool: ones-memset(i), mask(i, b0), mask(i, b1)
  SP : xg DMA one group ahead; og DMA after the last scale of a group

PSUM (8 banks): qk [64,2,2,T] x2 | v [128,4,65] x2 | o x2 | wei0+wei1 x1.
"""

import contextlib
import os
import sys

import numpy as np

for _p in ("/opt/trn_rl_repo",):
    if _p not in sys.path:
        sys.path.insert(0, _p)

B, T, C, H = 256, 256, 384, 64
N_CORES = 8
BPC = B // N_CORES  # batches per core
P = 128

LAST_RESULT = None  # BassKernelResults of the most recent run (for test.py)


def _build_nc(bpc=BPC, repeats=1, group=4):
    import concourse.bacc as bacc
    import concourse.mybir as mybir
    import concourse.tile as tile
    from concourse.masks import make_upper_triangular

    f32 = mybir.dt.float32
    f16 = mybir.dt.float16

    G = min(group, bpc)
    assert bpc % G == 0 and G % 2 == 0
    NG = bpc // G
    PPG = G // 2          # pairs per group
    NP = bpc // 2         # total pairs

    nc = bacc.Bacc("TRN2", target_bir_lowering=False, debug=False)

    xh = nc.dram_tensor("xh", [3, P, bpc, T], f16, kind="ExternalInput")
    wqk = nc.dram_tensor("wqk", [P, 3, 2, H], f16, kind="ExternalInput")
    wv = nc.dram_tensor("wv", [P, 3, H], f16, kind="ExternalInput")
    oh = nc.dram_tensor("oh", [P, bpc, 2, H], f16, kind="ExternalOutput")

    Exp = mybir.ActivationFunctionType.Exp
    mult = mybir.AluOpType.mult

    xh_r = xh.rearrange("c p b t -> p c b t")

    with tile.TileContext(nc) as tc:
        with (
            tc.tile_pool(name="consts", bufs=1) as consts,
            tc.tile_pool(name="xg", bufs=2) as xg_pool,
            tc.tile_pool(name="og", bufs=2) as og_pool,
            tc.tile_pool(name="sb", bufs=4) as sb,
            tc.tile_pool(name="ps_qk", bufs=1, space="PSUM") as ps_qk,
            tc.tile_pool(name="ps_v", bufs=2, space="PSUM") as ps_v,
            tc.tile_pool(name="ps_o", bufs=2, space="PSUM") as ps_o,
            tc.tile_pool(name="ps_wei", bufs=1, space="PSUM") as ps_wei,
        ):
            wqk_sb = consts.tile([P, 3, 2, H], f16)
            nc.sync.dma_start(wqk_sb, wqk[:])
            wv_sb = consts.tile([P, 3, H], f16)
            nc.sync.dma_start(wv_sb, wv[:])
            tri01 = consts.tile([P, P], f16)
            make_upper_triangular(nc, tri01, val=1.0, diag=True)

            rep_ctx = (
                tc.For_i(0, repeats, 1, hint_engines=(mybir.EngineType.PE,
                                                      mybir.EngineType.DVE,
                                                      mybir.EngineType.Activation,
                                                      mybir.EngineType.Pool,
                                                      mybir.EngineType.SP))
                if repeats > 1
                else contextlib.nullcontext()
            )
            with rep_ctx:
                state = {"xg": None, "og": None}

                def start_group(g):
                    xg_t = xg_pool.tile([P, 3, G, T], f16, tag="xg", name="xg")
                    nc.sync.dma_start(xg_t, xh_r[:, :, g * G:(g + 1) * G, :])
                    return xg_t

                xgs = {0: start_group(0)}
                prev = None  # deferred state of pair i-1

                def emit_tail(pv):
                    """AV (incl. ones-column rowsum) on PE for pair pv."""
                    o_ps = ps_o.tile([P, 2, 2, H + 1], f32, tag="o", name="o")
                    pv["o_ps"] = o_ps
                    for b in range(2):
                        p_sb = pv["p"][b]
                        v_sb = pv["v_sb"]
                        nc.tensor.matmul(
                            o_ps[:, b, 0, :], p_sb[:, 0:P], v_sb[:, 2 * b, :],
                            start=True, stop=True,
                        )
                        nc.tensor.matmul(
                            o_ps[:, b, 1, :], p_sb[:, P:T], v_sb[:, 2 * b, :],
                            start=True, stop=False,
                        )
                        nc.tensor.matmul(
                            o_ps[:, b, 1, :], p_sb[:, T:3 * P],
                            v_sb[:, 2 * b + 1, :],
                            start=False, stop=True,
                        )

                def emit_norm(pv):
                    o_ps = pv["o_ps"]
                    rinv = sb.tile([P, 2, 2], f32, tag="rinv")
                    nc.vector.reciprocal(rinv, o_ps[:, :, :, H])
                    nc.vector.tensor_tensor(
                        pv["og"][:, pv["j0"]:pv["j0"] + 2, :, :],
                        o_ps[:, :, :, 0:H],
                        rinv[:, :, :, None].to_broadcast((P, 2, 2, H)), mult,
                    )
                    if pv["last_in_group"]:
                        nc.sync.dma_start(
                            oh[:, pv["g"] * G:(pv["g"] + 1) * G, :, :],
                            pv["og"],
                        )

                def emit_wei(pv):
                    """weiT matmuls (PE) + exp (ACT) + mask (Pool)."""
                    qk_sb = pv["qk_sb"]
                    wei = []
                    for b in range(2):
                        wei_t = ps_wei.tile(
                            [P, 3 * P], f32, tag=f"wei{b}", name=f"wei{b}")
                        wei.append(wei_t)
                        nc.tensor.matmul(
                            wei_t[:, 0:T], qk_sb[:, 1, b, 0:P],
                            qk_sb[:, 0, b, :],
                            start=True, stop=True,
                        )
                        nc.tensor.matmul(
                            wei_t[:, T:3 * P], qk_sb[:, 1, b, P:T],
                            qk_sb[:, 0, b, P:T],
                            start=True, stop=True,
                        )
                    pv["wei"] = wei

                def emit_exp(pv):
                    ps = []
                    for b in range(2):
                        p_sb = sb.tile(
                            [P, 3 * P], f16, tag=f"p{b}", name=f"p{b}")
                        nc.scalar.activation(p_sb, pv["wei"][b], Exp)
                        p_diag = p_sb.rearrange(
                            "p (a q) -> p a q", q=P)[:, 0::2, :]
                        nc.gpsimd.tensor_tensor(
                            p_diag, p_diag,
                            tri01[:, None, :].to_broadcast((P, 2, P)), mult,
                        )
                        ps.append(p_sb)
                    pv["p"] = ps

                prev1 = None  # pair i-1: awaiting wei/exp/mask
                prev2 = None  # pair i-2: awaiting AV + normalization

                for idx in range(NP):
                    g, pj = divmod(idx, PPG)
                    j0 = pj * 2
                    xg = xgs.pop(g)
                    if pj == 0:
                        state["og"] = og_pool.tile(
                            [P, G, 2, H], f16, tag="og", name="og")
                        if g + 1 < NG:
                            xgs[g + 1] = start_group(g + 1)
                    og = state["og"]
                    if pj + 1 < PPG:
                        xgs[g] = xg

                    # ---- PE: separate q/k pair-wide projections + v ----
                    qk_ps = ps_qk.tile([H, 2, 2, T], f32, tag="qk")  # [w,b,t]
                    for w in range(2):
                        for c in range(3):
                            nc.tensor.matmul(
                                qk_ps[:, w, :, :], wqk_sb[:, c, w, :],
                                xg[:, c, j0:j0 + 2, :],
                                start=(c == 0), stop=(c == 2),
                            )
                    v_ps = ps_v.tile([P, 4, H], f32, tag="v")  # [(b i), h]
                    for b in range(2):
                        for i in range(2):
                            for c in range(3):
                                nc.tensor.matmul(
                                    v_ps[:, 2 * b + i, :],
                                    xg[:, c, j0 + b, i * P:(i + 1) * P],
                                    wv_sb[:, c, :],
                                    start=(c == 0), stop=(c == 2),
                                )

                    # ---- PE: deferred AV of pair i-2, then weiT of i-1
                    # (covers the qk/v copy latency of pair i) ----
                    if prev2 is not None:
                        emit_tail(prev2)
                    if prev1 is not None:
                        emit_wei(prev1)

                    # ---- copies for pair i ----
                    qk_sb = sb.tile([H, 2, 2, T], f16, tag="qk_sb")
                    nc.vector.tensor_copy(qk_sb, qk_ps)
                    v_sb = sb.tile([P, 4, H + 1], f16, tag="v_sb")
                    nc.gpsimd.memset(v_sb[:, :, H:H + 1], 1.0)
                    nc.scalar.copy(v_sb[:, :, 0:H], v_ps)

                    # ---- deferred normalization of i-2, exp/mask of i-1 ----
                    if prev2 is not None:
                        emit_norm(prev2)
                    if prev1 is not None:
                        emit_exp(prev1)

                    prev2 = prev1
                    prev1 = {
                        "qk_sb": qk_sb, "v_sb": v_sb,
                        "og": og, "j0": j0, "g": g,
                        "last_in_group": pj == PPG - 1,
                    }

                # drain the last two pairs
                if prev2 is not None:
                    emit_tail(prev2)
                emit_wei(prev1)
                if prev2 is not None:
                    emit_norm(prev2)
                emit_exp(prev1)
                emit_tail(prev1)
                emit_norm(prev1)

    nc.compile()
    return nc


def _prep_inputs(x, Wk, Wq, Wv):
    """Full inputs -> per-core in_maps with the DRAM layouts above."""
    x = np.asarray(x, dtype=np.float32)
    scale = np.float32(H) ** np.float32(-0.5)
    wq = np.asarray(Wq, dtype=np.float32) * scale
    wk = np.asarray(Wk, dtype=np.float32)
    wv = np.asarray(Wv, dtype=np.float32)
    # wqk[p, c, w, h]: w=0 -> Wq_scaled, w=1 -> Wk
    wqk_arr = np.stack(
        [wq.reshape(3, P, H), wk.reshape(3, P, H)], axis=2
    ).transpose(1, 0, 2, 3)
    wqk_arr = np.ascontiguousarray(wqk_arr.astype(np.float16))
    wv_arr = np.ascontiguousarray(
        wv.reshape(3, P, H).transpose(1, 0, 2).astype(np.float16)
    )
    in_maps = []
    for cid in range(N_CORES):
        xc = x[cid * BPC:(cid + 1) * BPC]  # [bpc, T, C]
        xh = xc.reshape(BPC, T, 3, P).transpose(2, 3, 0, 1)  # [3, P, bpc, T]
        in_maps.append({
            "xh": np.ascontiguousarray(xh.astype(np.float16)),
            "wqk": wqk_arr,
            "wv": wv_arr,
        })
    return in_maps


def _assemble_output(results):
    """Per-core oh [P, bpc, 2, H] fp16 -> full out [B, T, H] fp32."""
    outs = []
    for r in results:
        oh = np.asarray(r["oh"], dtype=np.float32)  # [P, bpc, 2, H]
        outs.append(oh.transpose(1, 2, 0, 3).reshape(BPC, T, H))
    return np.concatenate(outs, axis=0)


def kernel(x, Wk, Wq, Wv):
    global LAST_RESULT
    from concourse.bass_utils import run_bass_kernel_spmd

    in_maps = _prep_inputs(x, Wk, Wq, Wv)
    nc = _build_nc()
    trace = bool(int(os.environ.get("KERNEL_TRACE", "0")))
    res = run_bass_kernel_spmd(
        nc, in_maps, core_ids=list(range(N_CORES)), trace=trace
    )
    LAST_RESULT = res
    return _assemble_output(res.results)


# revision 6
# speedup vs baseline: 1.1545x; 1.1545x over previous
"""Causal single-head attention (B=256, T=256, C=384, H=64) on 8 trn2 cores.

Data-parallel over batch: each core computes bpc=32 batches independently.

v2 design (vs baseline): fp16 matmul dtype (1 cycle/row at any output size,
halves DMA bytes), transposed-softmax formulation (weiT = k^T q with s on
partitions -> no PE transposes at all), natural-layout v projection
(x-stationary), row sums via an appended ones-column in the AV matmul,
causal masking as a multiplicative 0/1 triangle on the Pool engine (SBUF
fp16), and group-batched DMAs (G batches per DMA) to amortize per-DMA
overheads (~625ns HWDGE + ~565ns SEQ each).

Per batch:
  qT[h,t], kT[h,t] = Wq/Wk stationary @ xT moving     (2x3 matmuls, [64,2,256] PSUM)
  v[t,h]           = xT-block stationary @ Wv moving  (6 matmuls, [128,2,64] PSUM)
  weiT[s,t]        = kT-block stationary @ qT moving  (2 matmuls, [128,384] PSUM:
                     cols 0:256 = s0 x all t, cols 256:384 = s1 x t1)
  p = exp(weiT) on ACT (PSUM->SBUF fp16, one instruction; logits ~N(0,1) so no
      max-subtraction needed), diagonal blocks masked by 0/1 upper-triangle
      multiply on Pool.
  out[t, 0:64] + rowsum[t] = p-block stationary @ [v|1] moving (3 matmuls)
  out scaled by 1/rowsum on the PSUM->SBUF copy (DVE for t0, ACT for t1).
"""

import contextlib
import os
import sys

import numpy as np

for _p in ("/opt/trn_rl_repo",):
    if _p not in sys.path:
        sys.path.insert(0, _p)

B, T, C, H = 256, 256, 384, 64
N_CORES = 8
BPC = B // N_CORES  # batches per core
P = 128

LAST_RESULT = None  # BassKernelResults of the most recent run (for test.py)


def _build_nc(bpc=BPC, repeats=1, group=8):
    import concourse.bacc as bacc
    import concourse.mybir as mybir
    import concourse.tile as tile
    from concourse.masks import make_upper_triangular

    f32 = mybir.dt.float32
    f16 = mybir.dt.float16

    G = min(group, bpc)
    assert bpc % G == 0
    NG = bpc // G

    nc = bacc.Bacc("TRN2", target_bir_lowering=False, debug=False)

    xh = nc.dram_tensor("xh", [3, P, bpc, T], f16, kind="ExternalInput")
    wqk = nc.dram_tensor("wqk", [P, 3, 2, H], f16, kind="ExternalInput")
    wv = nc.dram_tensor("wv", [P, 3, H], f16, kind="ExternalInput")
    oh = nc.dram_tensor("oh", [P, bpc, 2, H], f16, kind="ExternalOutput")

    Exp = mybir.ActivationFunctionType.Exp
    Copy = mybir.ActivationFunctionType.Copy
    mult = mybir.AluOpType.mult

    xh_r = xh.rearrange("c p b t -> p c b t")

    with tile.TileContext(nc) as tc:
        with (
            tc.tile_pool(name="consts", bufs=1) as consts,
            tc.tile_pool(name="xg", bufs=2) as xg_pool,
            tc.tile_pool(name="og", bufs=2) as og_pool,
            tc.tile_pool(name="sb", bufs=8) as sb,
            tc.tile_pool(name="ps_qk", bufs=2, space="PSUM") as ps_qk,
            tc.tile_pool(name="ps_v", bufs=2, space="PSUM") as ps_v,
            tc.tile_pool(name="ps_wei", bufs=2, space="PSUM") as ps_wei,
            tc.tile_pool(name="ps_o", bufs=2, space="PSUM") as ps_o,
        ):
            wqk_sb = consts.tile([P, 3, 2, H], f16)
            nc.sync.dma_start(wqk_sb, wqk[:])
            wv_sb = consts.tile([P, 3, H], f16)
            nc.sync.dma_start(wv_sb, wv[:])
            tri01 = consts.tile([P, P], f16)
            make_upper_triangular(nc, tri01, val=1.0, diag=True)

            rep_ctx = (
                tc.For_i(0, repeats, 1, hint_engines=(mybir.EngineType.PE,
                                                      mybir.EngineType.DVE,
                                                      mybir.EngineType.Activation,
                                                      mybir.EngineType.Pool,
                                                      mybir.EngineType.SP))
                if repeats > 1
                else contextlib.nullcontext()
            )
            with rep_ctx:
              for g in range(NG):
                xg = xg_pool.tile([P, 3, G, T], f16, tag="xg")
                nc.sync.dma_start(xg, xh_r[:, :, g * G:(g + 1) * G, :])
                og = og_pool.tile([P, G, 2, H], f16, tag="og")
                for j in range(G):
                    # ---- projections ----
                    qk_ps = ps_qk.tile([H, 2, T], f32, tag="qk")
                    for w in range(2):
                        for c in range(3):
                            nc.tensor.matmul(
                                qk_ps[:, w, :], wqk_sb[:, c, w, :], xg[:, c, j, :],
                                start=(c == 0), stop=(c == 2),
                            )
                    v_ps = ps_v.tile([P, 2, H], f32, tag="v")
                    for i in range(2):
                        for c in range(3):
                            nc.tensor.matmul(
                                v_ps[:, i, :],
                                xg[:, c, j, i * P:(i + 1) * P], wv_sb[:, c, :],
                                start=(c == 0), stop=(c == 2),
                            )
                    qk_sb = sb.tile([H, 2, T], f16, tag="qk_sb")
                    nc.vector.tensor_copy(qk_sb, qk_ps)
                    v_aug = sb.tile([P, 2, H + 1], f16, tag="v_aug")
                    nc.gpsimd.memset(v_aug[:, :, H:H + 1], 1.0)
                    nc.scalar.copy(v_aug[:, :, 0:H], v_ps)

                    # ---- weiT = k^T q, [s, t] with s on partitions ----
                    wei_ps = ps_wei.tile([P, 3 * P], f32, tag="wei")
                    nc.tensor.matmul(
                        wei_ps[:, 0:T], qk_sb[:, 1, 0:P], qk_sb[:, 0, :],
                        start=True, stop=True,
                    )
                    nc.tensor.matmul(
                        wei_ps[:, T:3 * P], qk_sb[:, 1, P:T], qk_sb[:, 0, P:T],
                        start=True, stop=True,
                    )

                    # ---- softmax numerator (no max subtraction) ----
                    p_sb = sb.tile([P, 3 * P], f16, tag="p")
                    nc.scalar.activation(p_sb, wei_ps, Exp)
                    # causal mask: zero strict-lower triangle of the two
                    # diagonal (s,t) blocks (cols 0:128 and 256:384) in one
                    # strided op
                    p_diag = p_sb.rearrange("p (a q) -> p a q", q=P)[:, 0::2, :]
                    nc.gpsimd.tensor_tensor(
                        p_diag, p_diag,
                        tri01[:, None, :].to_broadcast((P, 2, P)), mult,
                    )

                    # ---- out = p @ [v|1] ----
                    o_ps = ps_o.tile([P, 2, H + 1], f32, tag="o")
                    nc.tensor.matmul(
                        o_ps[:, 0, :], p_sb[:, 0:P], v_aug[:, 0, :],
                        start=True, stop=True,
                    )
                    nc.tensor.matmul(
                        o_ps[:, 1, :], p_sb[:, P:T], v_aug[:, 0, :],
                        start=True, stop=False,
                    )
                    nc.tensor.matmul(
                        o_ps[:, 1, :], p_sb[:, T:3 * P], v_aug[:, 1, :],
                        start=False, stop=True,
                    )

                    # ---- normalize by rowsum (col H of o_ps) ----
                    rinv = sb.tile([P, 2], f32, tag="rinv")
                    nc.vector.reciprocal(rinv, o_ps[:, :, H])
                    nc.vector.tensor_tensor(
                        og[:, j, 0, :], o_ps[:, 0, 0:H],
                        rinv[:, 0:1].to_broadcast((P, H)), mult,
                    )
                    nc.scalar.activation(
                        og[:, j, 1, :], o_ps[:, 1, 0:H], Copy, scale=rinv[:, 1:2]
                    )
                nc.scalar.dma_start(oh[:, g * G:(g + 1) * G, :, :], og)

    nc.compile()
    return nc


def _prep_inputs(x, Wk, Wq, Wv):
    """Full inputs -> per-core in_maps with the DRAM layouts above."""
    x = np.asarray(x, dtype=np.float32)
    scale = np.float32(H) ** np.float32(-0.5)
    wq = np.asarray(Wq, dtype=np.float32) * scale
    wk = np.asarray(Wk, dtype=np.float32)
    wv = np.asarray(Wv, dtype=np.float32)
    # wqk[p, c, w, h]
    wqk_arr = np.stack(
        [wq.reshape(3, P, H), wk.reshape(3, P, H)], axis=2
    ).transpose(1, 0, 2, 3)
    wqk_arr = np.ascontiguousarray(wqk_arr.astype(np.float16))
    wv_arr = np.ascontiguousarray(
        wv.reshape(3, P, H).transpose(1, 0, 2).astype(np.float16)
    )
    in_maps = []
    for cid in range(N_CORES):
        xc = x[cid * BPC:(cid + 1) * BPC]  # [bpc, T, C]
        xh = xc.reshape(BPC, T, 3, P).transpose(2, 3, 0, 1)  # [3, P, bpc, T]
        in_maps.append({
            "xh": np.ascontiguousarray(xh.astype(np.float16)),
            "wqk": wqk_arr,
            "wv": wv_arr,
        })
    return in_maps


def _assemble_output(results):
    """Per-core oh [P, bpc, 2, H] fp16 -> full out [B, T, H] fp32."""
    outs = []
    for r in results:
        oh = np.asarray(r["oh"], dtype=np.float32)  # [P, bpc, 2, H]
        outs.append(oh.transpose(1, 2, 0, 3).reshape(BPC, T, H))
    return np.concatenate(outs, axis=0)


def kernel(x, Wk, Wq, Wv):
    global LAST_RESULT
    from concourse.bass_utils import run_bass_kernel_spmd

    in_maps = _prep_inputs(x, Wk, Wq, Wv)
    nc = _build_nc()
    trace = bool(int(os.environ.get("KERNEL_TRACE", "0")))
    res = run_bass_kernel_spmd(
        nc, in_maps, core_ids=list(range(N_CORES)), trace=trace
    )
    LAST_RESULT = res
    return _assemble_output(res.results)


# revision 7
# speedup vs baseline: 1.2786x; 1.1075x over previous
"""Causal single-head attention (B=256, T=256, C=384, H=64) on 8 trn2 cores.

Data-parallel over batch: each core computes bpc=32 batches independently.

v2 design (vs baseline): fp16 matmul dtype (1 cycle/row at any output size,
halves DMA bytes), transposed-softmax formulation (weiT = k^T q with s on
partitions -> no PE transposes at all), natural-layout v projection
(x-stationary), row sums via an appended ones-column in the AV matmul,
causal masking as a multiplicative 0/1 triangle on the Pool engine (SBUF
fp16), and group-batched DMAs (G batches per DMA) to amortize per-DMA
overheads (~625ns HWDGE + ~565ns SEQ each).

Per batch:
  qT[h,t], kT[h,t] = Wq/Wk stationary @ xT moving     (2x3 matmuls, [64,2,256] PSUM)
  v[t,h]           = xT-block stationary @ Wv moving  (6 matmuls, [128,2,64] PSUM)
  weiT[s,t]        = kT-block stationary @ qT moving  (2 matmuls, [128,384] PSUM:
                     cols 0:256 = s0 x all t, cols 256:384 = s1 x t1)
  p = exp(weiT) on ACT (PSUM->SBUF fp16, one instruction; logits ~N(0,1) so no
      max-subtraction needed), diagonal blocks masked by 0/1 upper-triangle
      multiply on Pool.
  out[t, 0:64] + rowsum[t] = p-block stationary @ [v|1] moving (3 matmuls)
  out scaled by 1/rowsum on the PSUM->SBUF copy (DVE for t0, ACT for t1).
"""

import contextlib
import os
import sys

import numpy as np

for _p in ("/opt/trn_rl_repo",):
    if _p not in sys.path:
        sys.path.insert(0, _p)

B, T, C, H = 256, 256, 384, 64
N_CORES = 8
BPC = B // N_CORES  # batches per core
P = 128

LAST_RESULT = None  # BassKernelResults of the most recent run (for test.py)


def _build_nc(bpc=BPC, repeats=1, group=4):
    import concourse.bacc as bacc
    import concourse.mybir as mybir
    import concourse.tile as tile
    from concourse.masks import make_upper_triangular

    f32 = mybir.dt.float32
    f16 = mybir.dt.float16

    G = min(group, bpc)
    assert bpc % G == 0
    NG = bpc // G

    nc = bacc.Bacc("TRN2", target_bir_lowering=False, debug=False)

    xh = nc.dram_tensor("xh", [3, P, bpc, T], f16, kind="ExternalInput")
    wqk = nc.dram_tensor("wqk", [P, 3, 2, H], f16, kind="ExternalInput")
    wv = nc.dram_tensor("wv", [P, 3, H], f16, kind="ExternalInput")
    oh = nc.dram_tensor("oh", [P, bpc, 2, H], f16, kind="ExternalOutput")

    Exp = mybir.ActivationFunctionType.Exp
    Copy = mybir.ActivationFunctionType.Copy
    mult = mybir.AluOpType.mult

    xh_r = xh.rearrange("c p b t -> p c b t")

    with tile.TileContext(nc) as tc:
        with (
            tc.tile_pool(name="consts", bufs=1) as consts,
            tc.tile_pool(name="xg", bufs=2) as xg_pool,
            tc.tile_pool(name="og", bufs=2) as og_pool,
            tc.tile_pool(name="sb", bufs=8) as sb,
            tc.tile_pool(name="ps_qk", bufs=2, space="PSUM") as ps_qk,
            tc.tile_pool(name="ps_v", bufs=2, space="PSUM") as ps_v,
            tc.tile_pool(name="ps_wei", bufs=2, space="PSUM") as ps_wei,
            tc.tile_pool(name="ps_o", bufs=2, space="PSUM") as ps_o,
        ):
            wqk_sb = consts.tile([P, 3, 2, H], f16)
            nc.sync.dma_start(wqk_sb, wqk[:])
            wv_sb = consts.tile([P, 3, H], f16)
            nc.sync.dma_start(wv_sb, wv[:])
            tri01 = consts.tile([P, P], f16)
            make_upper_triangular(nc, tri01, val=1.0, diag=True)

            rep_ctx = (
                tc.For_i(0, repeats, 1, hint_engines=(mybir.EngineType.PE,
                                                      mybir.EngineType.DVE,
                                                      mybir.EngineType.Activation,
                                                      mybir.EngineType.Pool,
                                                      mybir.EngineType.SP))
                if repeats > 1
                else contextlib.nullcontext()
            )
            with rep_ctx:
              for g in range(NG):
                xg = xg_pool.tile([P, 3, G, T], f16, tag="xg")
                nc.sync.dma_start(xg, xh_r[:, :, g * G:(g + 1) * G, :])
                og = og_pool.tile([P, G, 2, H], f16, tag="og")
                for j in range(G):
                    # ---- projections ----
                    qk_ps = ps_qk.tile([H, 2, T], f32, tag="qk")
                    for w in range(2):
                        for c in range(3):
                            nc.tensor.matmul(
                                qk_ps[:, w, :], wqk_sb[:, c, w, :], xg[:, c, j, :],
                                start=(c == 0), stop=(c == 2),
                            )
                    v_ps = ps_v.tile([P, 2, H], f32, tag="v")
                    for i in range(2):
                        for c in range(3):
                            nc.tensor.matmul(
                                v_ps[:, i, :],
                                xg[:, c, j, i * P:(i + 1) * P], wv_sb[:, c, :],
                                start=(c == 0), stop=(c == 2),
                            )
                    qk_sb = sb.tile([H, 2, T], f16, tag="qk_sb")
                    nc.vector.tensor_copy(qk_sb, qk_ps)
                    v_aug = sb.tile([P, 2, H + 1], f16, tag="v_aug")
                    nc.gpsimd.memset(v_aug[:, :, H:H + 1], 1.0)
                    nc.scalar.copy(v_aug[:, :, 0:H], v_ps)

                    # ---- weiT = k^T q, [s, t] with s on partitions ----
                    wei_ps = ps_wei.tile([P, 3 * P], f32, tag="wei")
                    nc.tensor.matmul(
                        wei_ps[:, 0:T], qk_sb[:, 1, 0:P], qk_sb[:, 0, :],
                        start=True, stop=True,
                    )
                    nc.tensor.matmul(
                        wei_ps[:, T:3 * P], qk_sb[:, 1, P:T], qk_sb[:, 0, P:T],
                        start=True, stop=True,
                    )

                    # ---- softmax numerator (no max subtraction) ----
                    p_sb = sb.tile([P, 3 * P], f16, tag="p")
                    nc.scalar.activation(p_sb, wei_ps, Exp)
                    # causal mask: zero strict-lower triangle of the two
                    # diagonal (s,t) blocks (cols 0:128 and 256:384) in one
                    # strided op
                    p_diag = p_sb.rearrange("p (a q) -> p a q", q=P)[:, 0::2, :]
                    nc.gpsimd.tensor_tensor(
                        p_diag, p_diag,
                        tri01[:, None, :].to_broadcast((P, 2, P)), mult,
                    )

                    # ---- out = p @ [v|1] ----
                    o_ps = ps_o.tile([P, 2, H + 1], f32, tag="o")
                    nc.tensor.matmul(
                        o_ps[:, 0, :], p_sb[:, 0:P], v_aug[:, 0, :],
                        start=True, stop=True,
                    )
                    nc.tensor.matmul(
                        o_ps[:, 1, :], p_sb[:, P:T], v_aug[:, 0, :],
                        start=True, stop=False,
                    )
                    nc.tensor.matmul(
                        o_ps[:, 1, :], p_sb[:, T:3 * P], v_aug[:, 1, :],
                        start=False, stop=True,
                    )

                    # ---- normalize by rowsum (col H of o_ps) ----
                    rinv = sb.tile([P, 2], f32, tag="rinv")
                    nc.vector.reciprocal(rinv, o_ps[:, :, H])
                    nc.vector.tensor_tensor(
                        og[:, j, 0, :], o_ps[:, 0, 0:H],
                        rinv[:, 0:1].to_broadcast((P, H)), mult,
                    )
                    nc.scalar.activation(
                        og[:, j, 1, :], o_ps[:, 1, 0:H], Copy, scale=rinv[:, 1:2]
                    )
                nc.sync.dma_start(oh[:, g * G:(g + 1) * G, :, :], og)

    nc.compile()
    return nc


def _prep_inputs(x, Wk, Wq, Wv):
    """Full inputs -> per-core in_maps with the DRAM layouts above."""
    x = np.asarray(x, dtype=np.float32)
    scale = np.float32(H) ** np.float32(-0.5)
    wq = np.asarray(Wq, dtype=np.float32) * scale
    wk = np.asarray(Wk, dtype=np.float32)
    wv = np.asarray(Wv, dtype=np.float32)
    # wqk[p, c, w, h]
    wqk_arr = np.stack(
        [wq.reshape(3, P, H), wk.reshape(3, P, H)], axis=2
    ).transpose(1, 0, 2, 3)
    wqk_arr = np.ascontiguousarray(wqk_arr.astype(np.float16))
    wv_arr = np.ascontiguousarray(
        wv.reshape(3, P, H).transpose(1, 0, 2).astype(np.float16)
    )
    in_maps = []
    for cid in range(N_CORES):
        xc = x[cid * BPC:(cid + 1) * BPC]  # [bpc, T, C]
        xh = xc.reshape(BPC, T, 3, P).transpose(2, 3, 0, 1)  # [3, P, bpc, T]
        in_maps.append({
            "xh": np.ascontiguousarray(xh.astype(np.float16)),
            "wqk": wqk_arr,
            "wv": wv_arr,
        })
    return in_maps


def _assemble_output(results):
    """Per-core oh [P, bpc, 2, H] fp16 -> full out [B, T, H] fp32."""
    outs = []
    for r in results:
        oh = np.asarray(r["oh"], dtype=np.float32)  # [P, bpc, 2, H]
        outs.append(oh.transpose(1, 2, 0, 3).reshape(BPC, T, H))
    return np.concatenate(outs, axis=0)


def kernel(x, Wk, Wq, Wv):
    global LAST_RESULT
    from concourse.bass_utils import run_bass_kernel_spmd

    in_maps = _prep_inputs(x, Wk, Wq, Wv)
    nc = _build_nc()
    trace = bool(int(os.environ.get("KERNEL_TRACE", "0")))
    res = run_bass_kernel_spmd(
        nc, in_maps, core_ids=list(range(N_CORES)), trace=trace
    )
    LAST_RESULT = res
    return _assemble_output(res.results)
